# revision 3
# baseline (speedup 1.0000x reference)
"""DropPart masking kernel for Trainium2 (8 NeuronCores, data-parallel over batch).

Problem: x (64, 256, 96, 32) f32. For each sample n and channel-group g (8 groups
x 32 channels), a keypoint defines a keep-box; if roll[n,g] < 0.5 the group's
channels are zeroed outside the box, else passed through unchanged.

Strategy:
  - Host computes the tiny per-(n,g) masks (96x32 each) from key_pts/roll in
    exact f32 arithmetic matching the reference, cast to bf16 (0/1 exact).
  - Batch dim sharded 8 samples/core. Per core the Bass/Tile kernel streams x
    through SBUF in [128ch, 3072hw] tiles, expands the per-group masks to
    per-channel masks with a tiny one-hot matmul on the TensorEngine (PSUM),
    multiplies on the VectorEngine, and streams out. Program is input-
    independent (mask values are data), so one NEFF runs SPMD on all cores.
"""

import numpy as np
import ml_dtypes

import concourse.bass as bass
import concourse.bacc as bacc
import concourse.tile as tile
from concourse import mybir
from concourse.bass_utils import run_bass_kernel_spmd

N, C, H, W = 64, 256, 96, 32
GROUPS = 8
P_DROP = 0.5
HW = H * W          # 3072
CHS = C // GROUPS   # 32
N_CORES = 8
NPC = N // N_CORES  # samples per core = 8
ROWS = NPC * C      # x rows per core = 2048

_F32 = mybir.dt.float32
_BF16 = mybir.dt.bfloat16


def _build_module(reps: int = 1):
    nc = bacc.Bacc("TRN2", target_bir_lowering=False, debug=False)

    x_d = nc.dram_tensor("x", [ROWS, HW], _F32, kind="ExternalInput").ap()
    m_d = nc.dram_tensor("masks", [NPC * GROUPS, HW], _BF16, kind="ExternalInput").ap()
    e_d = nc.dram_tensor("eyes", [GROUPS, 2 * 128], _BF16, kind="ExternalInput").ap()
    o_d = nc.dram_tensor("out", [ROWS, HW], _F32, kind="ExternalOutput").ap()

    PS = 1536  # psum chunk: 3 banks; 2 chunks per 128-channel tile

    with tile.TileContext(nc) as tc:
        with (
            tc.tile_pool(name="consts", bufs=1) as consts,
            tc.tile_pool(name="mpool", bufs=3) as mpool,
            tc.tile_pool(name="xpool", bufs=6) as xpool,
            tc.tile_pool(name="psum", bufs=2, space="PSUM") as psum,
        ):
            eyes = consts.tile([GROUPS, 2 * 128], _BF16)
            nc.sync.dma_start(eyes[:], e_d[:])

            for _rep in range(reps):
                for s in range(NPC):
                    m = mpool.tile([GROUPS, HW], _BF16)
                    nc.sync.dma_start(m[:], m_d[s * GROUPS : (s + 1) * GROUPS, :])
                    for h in range(2):  # channel halves: 0-127, 128-255
                        r0 = s * C + h * 128
                        xt = xpool.tile([128, HW], _F32)
                        nc.sync.dma_start(xt[:], x_d[r0 : r0 + 128, :])
                        for q in range(HW // PS):
                            pt = psum.tile([128, PS], _F32)
                            for j in range(PS // 512):
                                col = q * PS + j * 512
                                nc.tensor.matmul(
                                    pt[:, j * 512 : (j + 1) * 512],
                                    eyes[:, h * 128 : (h + 1) * 128],
                                    m[:, col : col + 512],
                                    start=True,
                                    stop=True,
                                )
                            nc.vector.tensor_mul(
                                xt[:, q * PS : (q + 1) * PS],
                                xt[:, q * PS : (q + 1) * PS],
                                pt[:],
                            )
                        nc.sync.dma_start(o_d[r0 : r0 + 128, :], xt[:])

    nc.compile()
    return nc


_NC = None


def _get_module():
    global _NC
    if _NC is None:
        _NC = _build_module()
    return _NC


def _host_masks(key_pts: np.ndarray, roll: np.ndarray) -> np.ndarray:
    """Per-(n,g) masks [N, GROUPS, H*W] in {0,1}, f32 math exactly as reference."""
    s = int(0.25 * W)
    kx = (key_pts[:, :GROUPS, 0] * np.float32(W)).astype(np.float32)
    ky = (key_pts[:, :GROUPS, 1] * np.float32(H)).astype(np.float32)
    cond = (roll[:, :GROUPS] < np.float32(P_DROP)) & (kx >= 0) & (ky >= 0)

    bx = np.floor(np.maximum(kx - s, np.float32(0.0)))
    ex = np.floor(np.minimum(kx + s, np.float32(W)))
    by = np.floor(np.maximum(ky - s, np.float32(0.0)))
    ey = np.floor(np.minimum(ky + s, np.float32(H)))

    xs = np.arange(W, dtype=np.float32)
    ys = np.arange(H, dtype=np.float32)
    inx = (xs[None, None, :] >= bx[:, :, None]) & (xs[None, None, :] < ex[:, :, None])
    iny = (ys[None, None, :] >= by[:, :, None]) & (ys[None, None, :] < ey[:, :, None])
    box = iny[:, :, :, None] & inx[:, :, None, :]  # [N, G, H, W] bool

    mask = np.where(cond[:, :, None, None], box, True)
    return mask.reshape(N, GROUPS, HW).astype(np.float32)


def _host_eyes() -> np.ndarray:
    """One-hot channel->group expanders, [GROUPS, 2*128] bf16.
    Column block h (128 cols) maps channels h*128+m to group (h*128+m)//CHS."""
    e = np.zeros((GROUPS, 2, 128), dtype=np.float32)
    for h in range(2):
        for mcol in range(128):
            e[(h * 128 + mcol) // CHS, h, mcol] = 1.0
    return e.reshape(GROUPS, 2 * 128).astype(ml_dtypes.bfloat16)


def kernel(x: np.ndarray, key_pts: np.ndarray, roll: np.ndarray, **_kw) -> np.ndarray:
    x = np.ascontiguousarray(np.asarray(x, dtype=np.float32))
    key_pts = np.asarray(key_pts, dtype=np.float32)
    roll = np.asarray(roll, dtype=np.float32)

    masks = _host_masks(key_pts, roll).astype(ml_dtypes.bfloat16)
    eyes = _host_eyes()
    xr = x.reshape(N, C, HW)

    in_maps = []
    for c in range(N_CORES):
        sl = slice(c * NPC, (c + 1) * NPC)
        in_maps.append(
            {
                "x": np.ascontiguousarray(xr[sl]).reshape(ROWS, HW),
                "masks": np.ascontiguousarray(masks[sl]).reshape(NPC * GROUPS, HW),
                "eyes": eyes,
            }
        )

    nc = _get_module()
    res = run_bass_kernel_spmd(nc, in_maps, list(range(N_CORES))).results
    out = np.concatenate(
        [res[c]["out"].reshape(NPC, C, H, W) for c in range(N_CORES)], axis=0
    )
    return out


# revision 6
# speedup vs baseline: 3.0879x; 3.0879x over previous
"""DropPart masking kernel for Trainium2 (8 NeuronCores, data-parallel over batch).

Problem: x (64, 256, 96, 32) f32. For each sample n and channel-group g (8 groups
x 32 channels), a keypoint defines a keep-box; if roll[n,g] < 0.5 the group's
channels are zeroed outside the box, else passed through unchanged.

Strategy:
  - Host computes the tiny per-(n,g) masks (96x32 each) from key_pts/roll in
    exact f32 arithmetic matching the reference, cast to bf16 (0/1 exact).
  - Batch dim sharded 8 samples/core. Per core the Bass/Tile kernel streams x
    through SBUF in [128ch, 3072hw] tiles, expands the per-group masks to
    per-channel masks with a tiny one-hot matmul on the TensorEngine (PSUM),
    multiplies on the VectorEngine, and streams out. Program is input-
    independent (mask values are data), so one NEFF runs SPMD on all cores.
"""

import numpy as np
import ml_dtypes

import concourse.bass as bass
import concourse.bacc as bacc
import concourse.tile as tile
from concourse import mybir
from concourse.bass_utils import run_bass_kernel_spmd

N, C, H, W = 64, 256, 96, 32
GROUPS = 8
P_DROP = 0.5
HW = H * W          # 3072
CHS = C // GROUPS   # 32
N_CORES = 8
NPC = N // N_CORES  # samples per core = 8
ROWS = NPC * C      # x rows per core = 2048

_F32 = mybir.dt.float32
_BF16 = mybir.dt.bfloat16


def _build_module(reps: int = 1, loop_reps: int = 1):
    """loop_reps > 1 wraps the body in a For_i dynamic loop (benchmark only;
    adds ~2us back-edge per iteration)."""
    from contextlib import nullcontext

    nc = bacc.Bacc("TRN2", target_bir_lowering=False, debug=False)

    x_d = nc.dram_tensor("x", [ROWS, HW], _F32, kind="ExternalInput").ap()
    m_d = nc.dram_tensor("masks", [NPC * GROUPS, HW], _BF16, kind="ExternalInput").ap()
    e_d = nc.dram_tensor("eyes", [GROUPS, 2 * 128], _BF16, kind="ExternalInput").ap()
    o_d = nc.dram_tensor("out", [ROWS, HW], _F32, kind="ExternalOutput").ap()

    PS = 1536  # psum chunk: 3 banks; 2 chunks per 128-channel tile

    with tile.TileContext(nc) as tc:
        with (
            tc.tile_pool(name="consts", bufs=1) as consts,
            tc.tile_pool(name="mpool", bufs=3) as mpool,
            tc.tile_pool(name="xpool", bufs=6) as xpool,
            tc.tile_pool(name="psum", bufs=2, space="PSUM") as psum,
        ):
            eyes = consts.tile([GROUPS, 2 * 128], _BF16)
            nc.sync.dma_start(eyes[:], e_d[:])

            loop_cm = tc.For_i(0, loop_reps, 1) if loop_reps > 1 else nullcontext()
            with loop_cm:
                for _rep in range(reps):
                    for s in range(NPC):
                        m = mpool.tile([GROUPS, HW], _BF16)
                        nc.sync.dma_start(m[:], m_d[s * GROUPS : (s + 1) * GROUPS, :])
                        for h in range(2):  # channel halves: 0-127, 128-255
                            r0 = s * C + h * 128
                            xt = xpool.tile([128, HW], _F32)
                            nc.sync.dma_start(xt[:], x_d[r0 : r0 + 128, :])
                            for q in range(HW // PS):
                                pt = psum.tile([128, PS], _F32)
                                for j in range(PS // 512):
                                    col = q * PS + j * 512
                                    nc.tensor.matmul(
                                        pt[:, j * 512 : (j + 1) * 512],
                                        eyes[:, h * 128 : (h + 1) * 128],
                                        m[:, col : col + 512],
                                        start=True,
                                        stop=True,
                                    )
                                nc.vector.tensor_mul(
                                    xt[:, q * PS : (q + 1) * PS],
                                    xt[:, q * PS : (q + 1) * PS],
                                    pt[:],
                                )
                            nc.sync.dma_start(o_d[r0 : r0 + 128, :], xt[:])

    nc.compile()
    return nc


_NC = None


def _get_module():
    global _NC
    if _NC is None:
        _NC = _build_module()
    return _NC


def _host_masks(key_pts: np.ndarray, roll: np.ndarray) -> np.ndarray:
    """Per-(n,g) masks [N, GROUPS, H*W] in {0,1}, f32 math exactly as reference."""
    s = int(0.25 * W)
    kx = (key_pts[:, :GROUPS, 0] * np.float32(W)).astype(np.float32)
    ky = (key_pts[:, :GROUPS, 1] * np.float32(H)).astype(np.float32)
    cond = (roll[:, :GROUPS] < np.float32(P_DROP)) & (kx >= 0) & (ky >= 0)

    bx = np.floor(np.maximum(kx - s, np.float32(0.0)))
    ex = np.floor(np.minimum(kx + s, np.float32(W)))
    by = np.floor(np.maximum(ky - s, np.float32(0.0)))
    ey = np.floor(np.minimum(ky + s, np.float32(H)))

    xs = np.arange(W, dtype=np.float32)
    ys = np.arange(H, dtype=np.float32)
    inx = (xs[None, None, :] >= bx[:, :, None]) & (xs[None, None, :] < ex[:, :, None])
    iny = (ys[None, None, :] >= by[:, :, None]) & (ys[None, None, :] < ey[:, :, None])
    box = iny[:, :, :, None] & inx[:, :, None, :]  # [N, G, H, W] bool

    mask = np.where(cond[:, :, None, None], box, True)
    return mask.reshape(N, GROUPS, HW).astype(np.float32)


def _host_eyes() -> np.ndarray:
    """One-hot channel->group expanders, [GROUPS, 2*128] bf16.
    Column block h (128 cols) maps channels h*128+m to group (h*128+m)//CHS."""
    e = np.zeros((GROUPS, 2, 128), dtype=np.float32)
    for h in range(2):
        for mcol in range(128):
            e[(h * 128 + mcol) // CHS, h, mcol] = 1.0
    return e.reshape(GROUPS, 2 * 128).astype(ml_dtypes.bfloat16)


def kernel(x: np.ndarray, key_pts: np.ndarray, roll: np.ndarray, **_kw) -> np.ndarray:
    x = np.ascontiguousarray(np.asarray(x, dtype=np.float32))
    key_pts = np.asarray(key_pts, dtype=np.float32)
    roll = np.asarray(roll, dtype=np.float32)

    masks = _host_masks(key_pts, roll).astype(ml_dtypes.bfloat16)
    eyes = _host_eyes()
    xr = x.reshape(N, C, HW)

    in_maps = []
    for c in range(N_CORES):
        sl = slice(c * NPC, (c + 1) * NPC)
        in_maps.append(
            {
                "x": np.ascontiguousarray(xr[sl]).reshape(ROWS, HW),
                "masks": np.ascontiguousarray(masks[sl]).reshape(NPC * GROUPS, HW),
                "eyes": eyes,
            }
        )

    nc = _get_module()
    res = run_bass_kernel_spmd(nc, in_maps, list(range(N_CORES))).results
    out = np.concatenate(
        [res[c]["out"].reshape(NPC, C, H, W) for c in range(N_CORES)], axis=0
    )
    return out
